# revision 1
# baseline (speedup 1.0000x reference)
"""Trainium2 Bass kernel for nn_DifferentISLoss.

Math: the reference's scatter-adds fold away because the loss is a scalar.
With per-sample
    a_i = start[s_i0] + end[s_i,last] + sum_j B[s_ij, s_i,j+1]
    b_i = sum_j Bias[s_ij, s_i,j+1]
the loss is
    loss = -start[0] - end[-1] - sum_m (B+Bias)[m, m+1]
           + sum_i a_i*(a_i+b_i) / sum_i a_i

So the kernel is a pure gather-reduce: 8192 samples x 2047 pairs, each pair
needing (B, B+Bias) at one random (row, col). A stacked table
T[f] = (B.flat[f], (B+Bias).flat[f]) with appended start/end rows makes each
sample one stream of 2049 8-byte gathers whose two strided row-sums are a_i
and a_i+b_i directly.

Sharding: 1024 samples per core across 8 cores (data parallel over samples,
per the scatter->gather folding each core only produces 3 partial scalars);
host sums the 8 tiny [128, 24] outputs.

Mechanism: this stack's indirect DMA lowers correctly only in its
one-offset-per-partition form (128 random 8B rows per instruction), so the
kernel issues 2049 such gathers per 128-sample tile from the pool engine,
double-buffered against DVE index-math and row-sum reduces.
"""

import sys

for _p in ("/opt/trn_rl_repo",):
    if _p not in sys.path:
        sys.path.insert(0, _p)

from contextlib import ExitStack

import numpy as np

NW = 2048
NS = 8192
NCORES = 8
SHARD = NS // NCORES     # 1024 samples per core
TILES = SHARD // 128     # 8 tiles of 128 samples
KR = NW + 1              # 2049 gathers per sample: 2047 pairs + start + end
START_BASE = NW * NW
END_BASE = START_BASE + NW
ZERO_IDX = END_BASE + NW
TROWS = ZERO_IDX + 256

_NC_CACHE = {}


def _build_nc(detect_races=True):
    key = ("nc", detect_races)
    if key in _NC_CACHE:
        return _NC_CACHE[key]
    from concourse import bass, mybir

    f32 = mybir.dt.float32
    i32 = mybir.dt.int32
    i16 = mybir.dt.int16
    ADD = mybir.AluOpType.add
    MUL = mybir.AluOpType.mult
    XY = mybir.AxisListType.XY

    nc = bass.Bass(detect_race_conditions=detect_races)
    table_e = nc.declare_dram_parameter("table", [TROWS, 2], f32, isOutput=False)
    s16_e = nc.declare_dram_parameter("s16", [SHARD, NW], i16, isOutput=False)
    didx_e = nc.declare_dram_parameter("diag_idx", [128, 16], i32, isOutput=False)
    res_e = nc.declare_dram_parameter("res", [128, 24], f32, isOutput=True)

    ctx = ExitStack()
    sb = lambda name, shape, dt: ctx.enter_context(nc.sbuf_tensor(name, shape, dt))

    s16b = [sb(f"s16b{b}", [128, NW], i16) for b in range(2)]
    sf = sb("sf", [128, NW], f32)
    fidx = sb("fidx", [128, KR], f32)
    fi32b = [sb(f"fi32b{b}", [128, KR], i32) for b in range(2)]
    gb = [sb(f"g{b}", [128, KR * 2], f32) for b in range(2)]
    dix = sb("dix", [128, 16], i32)
    gd = sb("gd", [128, 32], f32)
    res = sb("res_sb", [128, 24], f32)

    def even_view(t):
        return t[:].rearrange("p (k two) -> p k two", two=2)[:, :, 0:1]

    def odd_view(t):
        return t[:].rearrange("p (k two) -> p k two", two=2)[:, :, 1:2]

    with (
        nc.Block() as block,
        nc.semaphore("sem_load0") as sem_load0,
        nc.semaphore("sem_load1") as sem_load1,
        nc.semaphore("sem_dix") as sem_dix,
        nc.semaphore("sem_idx") as sem_idx,
        nc.semaphore("sem_gat0") as sem_gat0,
        nc.semaphore("sem_gat1") as sem_gat1,
        nc.semaphore("sem_gatd") as sem_gatd,
        nc.semaphore("sem_red") as sem_red,
        nc.semaphore("sem_out") as sem_out,
    ):
        sem_load = [sem_load0, sem_load1]
        sem_gat = [sem_gat0, sem_gat1]

        @block.sync
        def _(sync: bass.BassEngine):
            sync.dma_start(out=s16b[0][:], in_=s16_e[0:128, :]).then_inc(sem_load[0], 16)
            sync.dma_start(out=dix[:], in_=didx_e[:]).then_inc(sem_dix, 16)
            sync.dma_start(out=s16b[1][:], in_=s16_e[128:256, :]).then_inc(sem_load[1], 16)
            for t in range(2, TILES):
                # buffer t%2 free once DVE consumed tile t-2's samples
                sync.wait_ge(sem_idx, t - 1)
                sync.dma_start(
                    out=s16b[t % 2][:], in_=s16_e[t * 128:(t + 1) * 128, :]
                ).then_inc(sem_load[t % 2], 16)
            sync.wait_ge(sem_red, TILES + 1)
            sync.dma_start(out=res_e[:], in_=res[:]).then_inc(sem_out, 16)
            sync.wait_ge(sem_out, 16)

        @block.vector
        def _(vec: bass.BassEngine):
            vec.memset(res[:, 18:24], 0.0)

            def reduce_tile(t):
                vec.wait_ge(sem_gat[t % 2], 16 * KR * (t // 2 + 1))
                g = gb[t % 2]
                vec.tensor_reduce(out=res[:, t:t + 1], in_=even_view(g), axis=XY, op=ADD)
                vec.tensor_reduce(
                    out=res[:, 8 + t:9 + t], in_=odd_view(g), axis=XY, op=ADD
                ).then_inc(sem_red, 1)

            for t in range(TILES):
                # ---- index math for tile t ----
                vec.wait_ge(sem_load[t % 2], 16 * (t // 2 + 1))
                vec.tensor_copy(out=sf[:], in_=s16b[t % 2][:])
                vec.scalar_tensor_tensor(
                    out=fidx[:, 0:NW - 1],
                    in0=sf[:, 0:NW - 1],
                    scalar=float(NW),
                    in1=sf[:, 1:NW],
                    op0=MUL,
                    op1=ADD,
                )
                vec.tensor_scalar_add(
                    out=fidx[:, NW - 1:NW], in0=sf[:, 0:1], scalar1=float(START_BASE)
                )
                vec.tensor_scalar_add(
                    out=fidx[:, NW:NW + 1], in0=sf[:, NW - 1:NW], scalar1=float(END_BASE)
                )
                if t >= 2:
                    # fi32 buffer reused from tile t-2: pool must be past it
                    vec.wait_ge(sem_gat[t % 2], 16 * KR * ((t - 2) // 2 + 1))
                vec.tensor_copy(out=fi32b[t % 2][:], in_=fidx[:]).then_inc(sem_idx, 1)
                # ---- reduces for tile t-1 ----
                if t >= 1:
                    reduce_tile(t - 1)
            reduce_tile(TILES - 1)
            # diag reduces
            vec.wait_ge(sem_gatd, 256)
            vec.tensor_reduce(out=res[:, 16:17], in_=even_view(gd), axis=XY, op=ADD)
            vec.tensor_reduce(
                out=res[:, 17:18], in_=odd_view(gd), axis=XY, op=ADD
            ).then_inc(sem_red, 1)

        @block.gpsimd
        def _(pool: bass.BassEngine):
            for t in range(TILES):
                pool.wait_ge(sem_idx, t + 1)
                if t >= 2:
                    # g buffer reuse: reduces of tile t-2 must be done
                    pool.wait_ge(sem_red, t - 1)
                fi = fi32b[t % 2]
                g = gb[t % 2]
                for k in range(KR):
                    # walrus requires sync info on every dynamic DMA
                    pool.indirect_dma_start(
                        out=g[:, 2 * k:2 * k + 2],
                        out_offset=None,
                        in_=table_e[:],
                        in_offset=bass.IndirectOffsetOnAxis(ap=fi[:, k:k + 1], axis=0),
                    ).then_inc(sem_gat[t % 2], 16)
            pool.wait_ge(sem_dix, 16)
            for k in range(16):
                pool.indirect_dma_start(
                    out=gd[:, 2 * k:2 * k + 2],
                    out_offset=None,
                    in_=table_e[:],
                    in_offset=bass.IndirectOffsetOnAxis(ap=dix[:, k:k + 1], axis=0),
                ).then_inc(sem_gatd, 16)

    ctx.close()
    _NC_CACHE[key] = nc
    return nc


def _stage_inputs(bigram, start, end, bigram_bias, samples):
    bigram = np.asarray(bigram, dtype=np.float32)
    start = np.asarray(start, dtype=np.float32)
    end = np.asarray(end, dtype=np.float32)
    bigram_bias = np.asarray(bigram_bias, dtype=np.float32)
    samples = np.asarray(samples)

    table = np.zeros((TROWS, 2), np.float32)
    tmain = table[: NW * NW].reshape(NW, NW, 2)
    tmain[:, :, 0] = bigram
    tmain[:, :, 1] = bigram + bigram_bias
    table[START_BASE:START_BASE + NW, 0] = start
    table[START_BASE:START_BASE + NW, 1] = start
    table[END_BASE:END_BASE + NW, 0] = end
    table[END_BASE:END_BASE + NW, 1] = end
    # ZERO_IDX row (and the rest of the pad) stays (0, 0)

    s16 = np.ascontiguousarray(samples.astype(np.int16))

    m = np.arange(128 * 16)
    dflat = np.where(m < NW - 1, m * NW + m + 1, ZERO_IDX).astype(np.int32)
    diag_idx = dflat.reshape(128, 16)

    in_maps = [
        {
            "table": table,
            "s16": s16[k * SHARD:(k + 1) * SHARD],
            "diag_idx": diag_idx,
        }
        for k in range(NCORES)
    ]
    return in_maps, start, end


def _combine(results, start, end):
    even = np.concatenate([r["res"][:, 0:8] for r in results]).astype(np.float64)
    odd = np.concatenate([r["res"][:, 8:16] for r in results]).astype(np.float64)
    S = even.sum()
    N = (even * odd).sum()
    D = results[0]["res"][:, 17].astype(np.float64).sum()
    loss = -float(start[0]) - float(end[-1]) - D + N / S
    return np.asarray(loss, dtype=np.float32)


def _run(inputs, trace=False, **kw):
    from concourse.bass_utils import run_bass_kernel_spmd

    nc = _build_nc()
    in_maps, start, end = _stage_inputs(**inputs)
    out = run_bass_kernel_spmd(nc, in_maps, list(range(NCORES)), trace=trace, **kw)
    return _combine(out.results, start, end), out


def kernel(bigram, start, end, bigram_bias, samples):
    loss, _ = _run(
        dict(bigram=bigram, start=start, end=end, bigram_bias=bigram_bias, samples=samples)
    )
    return loss



# revision 2
# speedup vs baseline: 1.0086x; 1.0086x over previous
"""Trainium2 Bass kernel for nn_DifferentISLoss.

Math: the reference's scatter-adds fold away because the loss is a scalar.
With per-sample
    a_i = start[s_i0] + end[s_i,last] + sum_j B[s_ij, s_i,j+1]
    b_i = sum_j Bias[s_ij, s_i,j+1]
the loss is
    loss = -start[0] - end[-1] - sum_m (B+Bias)[m, m+1]
           + sum_i a_i*(a_i+b_i) / sum_i a_i

So the kernel is a pure gather-reduce: 8192 samples x 2047 pairs, each pair
needing (B, B+Bias) at one random (row, col). A stacked table
T[f] = (B.flat[f], (B+Bias).flat[f]) with appended start/end rows makes each
sample one stream of 2049 8-byte gathers whose two strided row-sums are a_i
and a_i+b_i directly.

Sharding: 1024 samples per core across 8 cores (data parallel over samples,
per the scatter->gather folding each core only produces 3 partial scalars);
host sums the 8 tiny [128, 24] outputs.

Mechanism: this stack's indirect DMA lowers correctly only in its
one-offset-per-partition form (128 random 8B rows per instruction), so the
kernel issues 2049 such gathers per 128-sample tile from the pool engine,
double-buffered against DVE index-math and row-sum reduces.
"""

import sys

for _p in ("/opt/trn_rl_repo",):
    if _p not in sys.path:
        sys.path.insert(0, _p)

from contextlib import ExitStack

import numpy as np

NW = 2048
NS = 8192
NCORES = 8
SHARD = NS // NCORES     # 1024 samples per core
TILES = SHARD // 128     # 8 tiles of 128 samples
KR = NW + 1              # 2049 gathers per sample: 2047 pairs + start + end
START_BASE = NW * NW
END_BASE = START_BASE + NW
ZERO_IDX = END_BASE + NW
TROWS = ZERO_IDX + 256

_NC_CACHE = {}


def _build_nc(detect_races=True):
    key = ("nc", detect_races)
    if key in _NC_CACHE:
        return _NC_CACHE[key]
    from concourse import bass, mybir

    f32 = mybir.dt.float32
    i32 = mybir.dt.int32
    i16 = mybir.dt.int16
    ADD = mybir.AluOpType.add
    MUL = mybir.AluOpType.mult
    XY = mybir.AxisListType.XY

    nc = bass.Bass(detect_race_conditions=detect_races, num_swdge_queues=4)
    table_e = nc.declare_dram_parameter("table", [TROWS, 2], f32, isOutput=False)
    s16_e = nc.declare_dram_parameter("s16", [SHARD, NW], i16, isOutput=False)
    didx_e = nc.declare_dram_parameter("diag_idx", [128, 16], i32, isOutput=False)
    res_e = nc.declare_dram_parameter("res", [128, 24], f32, isOutput=True)

    ctx = ExitStack()
    sb = lambda name, shape, dt: ctx.enter_context(nc.sbuf_tensor(name, shape, dt))

    s16b = [sb(f"s16b{b}", [128, NW], i16) for b in range(2)]
    sf = sb("sf", [128, NW], f32)
    fidx = sb("fidx", [128, KR], f32)
    fi32b = [sb(f"fi32b{b}", [128, KR], i32) for b in range(2)]
    gb = [sb(f"g{b}", [128, KR * 2], f32) for b in range(2)]
    dix = sb("dix", [128, 16], i32)
    gd = sb("gd", [128, 32], f32)
    res = sb("res_sb", [128, 24], f32)

    def even_view(t):
        return t[:].rearrange("p (k two) -> p k two", two=2)[:, :, 0:1]

    def odd_view(t):
        return t[:].rearrange("p (k two) -> p k two", two=2)[:, :, 1:2]

    with (
        nc.Block() as block,
        nc.semaphore("sem_load0") as sem_load0,
        nc.semaphore("sem_load1") as sem_load1,
        nc.semaphore("sem_dix") as sem_dix,
        nc.semaphore("sem_idx") as sem_idx,
        nc.semaphore("sg00") as sg00,
        nc.semaphore("sg01") as sg01,
        nc.semaphore("sg02") as sg02,
        nc.semaphore("sg03") as sg03,
        nc.semaphore("sg10") as sg10,
        nc.semaphore("sg11") as sg11,
        nc.semaphore("sg12") as sg12,
        nc.semaphore("sg13") as sg13,
        nc.semaphore("sem_gatd") as sem_gatd,
        nc.semaphore("sem_red") as sem_red,
        nc.semaphore("sem_out") as sem_out,
    ):
        sem_load = [sem_load0, sem_load1]
        sem_gat = [[sg00, sg01, sg02, sg03], [sg10, sg11, sg12, sg13]]
        # gathers per tile per queue: k % 4 == q over k in [0, KR)
        CNT = [KR - 3 * (KR // 4), KR // 4, KR // 4, KR // 4]

        @block.sync
        def _(sync: bass.BassEngine):
            sync.dma_start(out=s16b[0][:], in_=s16_e[0:128, :]).then_inc(sem_load[0], 16)
            sync.dma_start(out=dix[:], in_=didx_e[:]).then_inc(sem_dix, 16)
            sync.dma_start(out=s16b[1][:], in_=s16_e[128:256, :]).then_inc(sem_load[1], 16)
            for t in range(2, TILES):
                # buffer t%2 free once DVE consumed tile t-2's samples
                sync.wait_ge(sem_idx, t - 1)
                sync.dma_start(
                    out=s16b[t % 2][:], in_=s16_e[t * 128:(t + 1) * 128, :]
                ).then_inc(sem_load[t % 2], 16)
            sync.wait_ge(sem_red, TILES + 1)
            sync.dma_start(out=res_e[:], in_=res[:]).then_inc(sem_out, 16)
            sync.wait_ge(sem_out, 16)

        @block.vector
        def _(vec: bass.BassEngine):
            vec.memset(res[:, 18:24], 0.0)

            def reduce_tile(t):
                for q in range(4):
                    vec.wait_ge(sem_gat[t % 2][q], 16 * CNT[q] * (t // 2 + 1))
                g = gb[t % 2]
                vec.tensor_reduce(out=res[:, t:t + 1], in_=even_view(g), axis=XY, op=ADD)
                vec.tensor_reduce(
                    out=res[:, 8 + t:9 + t], in_=odd_view(g), axis=XY, op=ADD
                ).then_inc(sem_red, 1)

            for t in range(TILES):
                # ---- index math for tile t ----
                vec.wait_ge(sem_load[t % 2], 16 * (t // 2 + 1))
                vec.tensor_copy(out=sf[:], in_=s16b[t % 2][:])
                vec.scalar_tensor_tensor(
                    out=fidx[:, 0:NW - 1],
                    in0=sf[:, 0:NW - 1],
                    scalar=float(NW),
                    in1=sf[:, 1:NW],
                    op0=MUL,
                    op1=ADD,
                )
                vec.tensor_scalar_add(
                    out=fidx[:, NW - 1:NW], in0=sf[:, 0:1], scalar1=float(START_BASE)
                )
                vec.tensor_scalar_add(
                    out=fidx[:, NW:NW + 1], in0=sf[:, NW - 1:NW], scalar1=float(END_BASE)
                )
                if t >= 2:
                    # fi32 buffer reused from tile t-2: pool must be past it
                    for q in range(4):
                        vec.wait_ge(sem_gat[t % 2][q], 16 * CNT[q] * ((t - 2) // 2 + 1))
                vec.tensor_copy(out=fi32b[t % 2][:], in_=fidx[:]).then_inc(sem_idx, 1)
                # ---- reduces for tile t-1 ----
                if t >= 1:
                    reduce_tile(t - 1)
            reduce_tile(TILES - 1)
            # diag reduces
            vec.wait_ge(sem_gatd, 256)
            vec.tensor_reduce(out=res[:, 16:17], in_=even_view(gd), axis=XY, op=ADD)
            vec.tensor_reduce(
                out=res[:, 17:18], in_=odd_view(gd), axis=XY, op=ADD
            ).then_inc(sem_red, 1)

        @block.gpsimd
        def _(pool: bass.BassEngine):
            for t in range(TILES):
                pool.wait_ge(sem_idx, t + 1)
                if t >= 2:
                    # g buffer reuse: reduces of tile t-2 must be done
                    pool.wait_ge(sem_red, t - 1)
                fi = fi32b[t % 2]
                g = gb[t % 2]
                for k in range(KR):
                    # walrus requires sync info on every dynamic DMA; queue
                    # k%4 assigned post-build (4 SWDGE queues = 4 Q7 pairs)
                    pool.indirect_dma_start(
                        out=g[:, 2 * k:2 * k + 2],
                        out_offset=None,
                        in_=table_e[:],
                        in_offset=bass.IndirectOffsetOnAxis(ap=fi[:, k:k + 1], axis=0),
                    ).then_inc(sem_gat[t % 2][k % 4], 16)
            pool.wait_ge(sem_dix, 16)
            for k in range(16):
                pool.indirect_dma_start(
                    out=gd[:, 2 * k:2 * k + 2],
                    out_offset=None,
                    in_=table_e[:],
                    in_offset=bass.IndirectOffsetOnAxis(ap=dix[:, k:k + 1], axis=0),
                ).then_inc(sem_gatd, 16)

    ctx.close()
    # round-robin the tile gathers over the 4 SWDGE queues (emission order:
    # TILES*KR tile gathers, then 16 diag gathers which stay on queue 0)
    n = 0
    for f in nc.m.functions:
        for b in f.blocks:
            for ins in b.instructions:
                if isinstance(ins, mybir.InstDMACopy) and str(ins.queue or "").startswith("qPoolDynamic"):
                    if n < TILES * KR:
                        q = n % KR % 4
                        if q:
                            ins.queue = f"qPoolDynamic{q}"
                    n += 1
    assert n == TILES * KR + 16, n
    _NC_CACHE[key] = nc
    return nc


def _stage_inputs(bigram, start, end, bigram_bias, samples):
    bigram = np.asarray(bigram, dtype=np.float32)
    start = np.asarray(start, dtype=np.float32)
    end = np.asarray(end, dtype=np.float32)
    bigram_bias = np.asarray(bigram_bias, dtype=np.float32)
    samples = np.asarray(samples)

    table = np.zeros((TROWS, 2), np.float32)
    tmain = table[: NW * NW].reshape(NW, NW, 2)
    tmain[:, :, 0] = bigram
    tmain[:, :, 1] = bigram + bigram_bias
    table[START_BASE:START_BASE + NW, 0] = start
    table[START_BASE:START_BASE + NW, 1] = start
    table[END_BASE:END_BASE + NW, 0] = end
    table[END_BASE:END_BASE + NW, 1] = end
    # ZERO_IDX row (and the rest of the pad) stays (0, 0)

    s16 = np.ascontiguousarray(samples.astype(np.int16))

    m = np.arange(128 * 16)
    dflat = np.where(m < NW - 1, m * NW + m + 1, ZERO_IDX).astype(np.int32)
    diag_idx = dflat.reshape(128, 16)

    in_maps = [
        {
            "table": table,
            "s16": s16[k * SHARD:(k + 1) * SHARD],
            "diag_idx": diag_idx,
        }
        for k in range(NCORES)
    ]
    return in_maps, start, end


def _combine(results, start, end):
    even = np.concatenate([r["res"][:, 0:8] for r in results]).astype(np.float64)
    odd = np.concatenate([r["res"][:, 8:16] for r in results]).astype(np.float64)
    S = even.sum()
    N = (even * odd).sum()
    D = results[0]["res"][:, 17].astype(np.float64).sum()
    loss = -float(start[0]) - float(end[-1]) - D + N / S
    return np.asarray(loss, dtype=np.float32)


def _run(inputs, trace=False, **kw):
    from concourse.bass_utils import run_bass_kernel_spmd

    nc = _build_nc()
    in_maps, start, end = _stage_inputs(**inputs)
    out = run_bass_kernel_spmd(nc, in_maps, list(range(NCORES)), trace=trace, **kw)
    return _combine(out.results, start, end), out


def kernel(bigram, start, end, bigram_bias, samples):
    loss, _ = _run(
        dict(bigram=bigram, start=start, end=end, bigram_bias=bigram_bias, samples=samples)
    )
    return loss

